# revision 12
# baseline (speedup 1.0000x reference)
"""Causal attention kernel for Trainium2 (Bass/Tile), 8-core data-parallel.

Problem: x [8, 2048, 1024] f32; W_query/W_key/W_value [1024, 1024] f32.
    q = x @ Wq; k = x @ Wk; v = x @ Wv       (per batch element)
    out = softmax(causal(q k^T) / 32) @ v

Sharding: batch dim (8) across the 8 NeuronCores, one batch element per
core; each core runs the identical single-core program on its slice.

All heavy matmuls run in fp8 e4m3 with MatmulPerfMode.DoubleRow, which
contracts 256 rows per instruction at 0.5 cycles/moving-row (4x the
fp32r full rate).  fp8's ~2.6% elementwise quantization error is kept
out of the signal path with 2-term hi/lo splits (residual also in fp8):

  x  ~ x_hi + x_lo            W' = 32*W ~ W_hi + W_lo  (scaled out of
                                                        e4m3 subnormals)
  proj (3 terms):  p = x_hi@W_hi + x_lo@W_hi + x_hi@W_lo   (f32 PSUM)
  qT, kT = fp8(p)  (single; the only uncompensated quantization -
                    it only perturbs logits, ~1.1% output error)
  v ~ v_hi + v_lo
  scores computed TRANSPOSED: S^T[m,n] = kT-block (stationary) x qT
  (moving), so exp(S^T) IS P^T - no PE transposes of P needed.
  P ~ P_hi + P_lo (from bf16 exp staging)
  AV (3 terms): O = P_hi@v_hi + P_lo@v_hi + P_hi@v_lo    (f32 PSUM)
  den via tiny fp8 ones-matmuls accumulated alongside; out = O/(32 den)

Measured end-to-end (numpy bit-sim of this exact arithmetic):
rel err ~1.2e-2.  The exp scale is 1/(32*32*32): 1/sqrt(d) plus the
two W scalings.
"""

import os

import numpy as np

# Defensive: recover wedged cores at NRT/PJRT init (no-op on healthy devices).
os.environ.setdefault("NEURON_RT_RESET_CORES", "1")

import concourse.tile as tile
import concourse.mybir as mybir
from concourse import bacc, bass_utils
from concourse.masks import make_identity

F32 = mybir.dt.float32
F32R = mybir.dt.float32r
F8 = mybir.dt.float8e4
BF16 = mybir.dt.bfloat16
EXP = mybir.ActivationFunctionType.Exp
DR = mybir.MatmulPerfMode.DoubleRow
MULT = mybir.AluOpType.mult
SUB = mybir.AluOpType.subtract

NTOK = 2048      # tokens per batch element (= per core)
D = 1024         # d_in = d_out
P = 128          # partitions
DC = D // P      # 8 d-chunks
NBLK = NTOK // P     # 16 token blocks
NJ = NTOK // 512     # 4 query chunks of 512
NEG = -1.0e9
ESCALE = 1.0 / (32.0 * 32.0 * 32.0)  # exp scale: 1/sqrt(d) and two 32x W scales
WSCALE = 32.0


def build_program():
    nc = bacc.Bacc("TRN2", target_bir_lowering=False, debug=False,
                   num_devices=8)
    x = nc.dram_tensor("x", [NTOK, D], F32, kind="ExternalInput").ap()
    wq = nc.dram_tensor("W_query", [D, D], F32, kind="ExternalInput").ap()
    wk = nc.dram_tensor("W_key", [D, D], F32, kind="ExternalInput").ap()
    wv = nc.dram_tensor("W_value", [D, D], F32, kind="ExternalInput").ap()
    out = nc.dram_tensor("out", [NTOK, D], F32, kind="ExternalOutput").ap()

    with tile.TileContext(nc) as tc:
        _emit(nc, tc, x, wq, wk, wv, out)
    nc.compile()
    return nc


def _emit(nc, tc, x, wq, wk, wv, out):
    const = tc.alloc_tile_pool(name="const", bufs=1)
    res = tc.alloc_tile_pool(name="res", bufs=1)

    # constants
    id32 = const.tile([P, P], F32, tag="id32")
    make_identity(nc, id32)
    id32r = const.tile([P, P], F32R, tag="id32r")
    nc.vector.tensor_copy(id32r, id32)
    # additive causal triangle for transposed scores: T[p, f] = 0 where
    # f >= p (valid, n >= m), NEG where f < p.
    tri = const.tile([P, P], F32, tag="tri")
    nc.vector.memset(tri, 0.0)
    nc.gpsimd.affine_select(
        out=tri, in_=tri, compare_op=mybir.AluOpType.is_ge, fill=NEG,
        base=0, pattern=[[1, P]], channel_multiplier=-1)
    ones8 = const.tile([P, 2, 2], F8, tag="ones8")
    nc.vector.memset(ones8, 1.0)

    # fp8 residents
    xh = res.tile([P, DC, NTOK], F8, tag="xh")   # [d_in%128, dc, tok]
    xl = res.tile([P, DC, NTOK], F8, tag="xl")
    kT = res.tile([P, DC, NTOK], F8, tag="kT")   # [d_out%128, dc, m]
    qT = res.tile([P, DC, NTOK], F8, tag="qT")   # [d_out%128, dc, n]
    vh = res.tile([P, NBLK, D], F8, tag="vh")    # [m%128, mblock, d]
    vl = res.tile([P, NBLK, D], F8, tag="vl")
    wkh = res.tile([P, DC, D], F8, tag="wkh")    # [d_in%128, dc, d_out]
    wkl = res.tile([P, DC, D], F8, tag="wkl")

    # ---------------- phase 1: projections ----------------
    # x/out DMAs issue from SP; W DMAs from ACT so neither stream blocks
    # the other at the issue queue.
    # psp outlives phase 1 (reused for the transposed-score PSUM tiles),
    # so it is allocated first; pools are stack-ordered on release.
    psp = tc.alloc_tile_pool(name="psp", bufs=2, space="PSUM")
    xpool = tc.alloc_tile_pool(name="xpool", bufs=4)    # x f32 staging
    wfpool = tc.alloc_tile_pool(name="wfpool", bufs=3)  # W f32 staging (k/q)
    wvfpool = tc.alloc_tile_pool(name="wvfpool", bufs=2)  # W f32 staging (v)
    wqpool = tc.alloc_tile_pool(name="wqpool", bufs=3)  # Wq fp8 panels
    wvpool = tc.alloc_tile_pool(name="wvpool", bufs=2)  # Wv fp8 quarters
    pst = tc.alloc_tile_pool(name="pst", bufs=3, space="PSUM")

    def dma_w_panel(w_ap, pc):
        wf = wfpool.tile([P, DC, P], F32, tag="wf")
        nc.scalar.dma_start(
            out=wf,
            in_=w_ap[:, pc * P:(pc + 1) * P].rearrange("(c p) f -> p c f", p=P))
        return wf

    def quant_w(wf, hi_ap, lo_ap):
        # hi = fp8(32*W); lo = fp8(32*W - hi)
        nc.scalar.mul(hi_ap, wf, WSCALE)
        nc.vector.scalar_tensor_tensor(
            out=lo_ap, in0=wf, scalar=WSCALE, in1=hi_ap, op0=MULT, op1=SUB)

    def transpose_block(tbg):
        # x token block [128, 1024] -> xT hi/lo [:, :, tbg*128:+128]
        xt = xpool.tile([P, D], F32R, tag="xt")
        nc.sync.dma_start(out=xt, in_=x[tbg * P:(tbg + 1) * P, :].bitcast(F32R))
        for g in range(2):
            trp = pst.tile([P, 512], F32R, tag="trp")
            for b4 in range(4):
                nc.tensor.transpose(
                    trp[:, b4 * P:(b4 + 1) * P],
                    xt[:, (g * 4 + b4) * P:(g * 4 + b4 + 1) * P], id32r)
            src = trp.bitcast(F32).rearrange("p (b f) -> p b f", b=4)
            dst_h = xh[:, g * 4:(g + 1) * 4, tbg * P:(tbg + 1) * P]
            dst_l = xl[:, g * 4:(g + 1) * 4, tbg * P:(tbg + 1) * P]
            nc.scalar.copy(dst_h, src)
            nc.vector.tensor_sub(dst_l, src, dst_h)

    def proj_kq(dst, whi, wlo, pc, jc, pslice, requant_pool):
        # one [d_out 128, tok 512] panel-chunk of q/k projection
        ps = psp.tile([P, 512], F32, tag="psp")
        terms = ((whi, xh), (whi, xl), (wlo, xh))
        n = 0
        for wt, xt_ in terms:
            for dp in range(4):
                nc.tensor.matmul(
                    ps, wt[:, 2 * dp:2 * dp + 2, pslice],
                    xt_[:, 2 * dp:2 * dp + 2, jc * 512:(jc + 1) * 512],
                    start=(n == 0), stop=(n == 11), perf_mode=DR)
                n += 1
        if requant_pool:
            nc.gpsimd.tensor_copy(dst[:, pc, jc * 512:(jc + 1) * 512], ps)
        else:
            nc.scalar.copy(dst[:, pc, jc * 512:(jc + 1) * 512], ps)

    # k-projection: chunk-outer, interleaved with the x transposes for the
    # next chunk, so PE stays fed from the first x tile onward.  Wk panels
    # are quantized just-in-time on first use (jc == 0).
    for jc in range(NJ):
        for tbg in range(4 * jc, 4 * jc + 4):
            transpose_block(tbg)
        if jc == 0:
            wk_f32 = [dma_w_panel(wk, pc) for pc in range(3)]
        for pc in range(DC):
            if jc == 0:
                if pc + 3 < DC:
                    wk_f32.append(dma_w_panel(wk, pc + 3))
                quant_w(wk_f32[pc], wkh[:, :, pc * P:(pc + 1) * P],
                        wkl[:, :, pc * P:(pc + 1) * P])
            proj_kq(kT, wkh, wkl, pc, jc,
                    slice(pc * P, (pc + 1) * P), requant_pool=True)

    # q-proj: stream Wq panels with one-panel prefetch
    def load_wq(pc):
        wf = dma_w_panel(wq, pc)
        wqh = wqpool.tile([P, DC, P], F8, tag="wqh", name="wqh")
        wql = wqpool.tile([P, DC, P], F8, tag="wql", name="wql")
        quant_w(wf, wqh, wql)
        return wqh, wql

    nxt = load_wq(0)
    for pc in range(DC):
        cur, nxt = nxt, (load_wq(pc + 1) if pc + 1 < DC else None)
        for jc in range(NJ):
            proj_kq(qT, cur[0], cur[1], pc, jc,
                    slice(0, P), requant_pool=False)

    # v-proj: stream Wv in 256-wide d_out quarters with prefetch;
    # v needs a hi+lo split (ACT hi, Pool lo-sub)
    def load_wv(vq):
        wvf = wvfpool.tile([P, DC, 256], F32, tag="wvf")
        nc.scalar.dma_start(
            out=wvf,
            in_=wv[:, vq * 256:(vq + 1) * 256].rearrange("(c p) f -> p c f", p=P))
        wvh_ = wvpool.tile([P, DC, 256], F8, tag="wvh", name="wvh")
        wvl_ = wvpool.tile([P, DC, 256], F8, tag="wvl", name="wvl")
        quant_w(wvf, wvh_, wvl_)
        return wvh_, wvl_

    nxtv = load_wv(0)
    for vq in range(4):
        (wvh_, wvl_), nxtv = nxtv, (load_wv(vq + 1) if vq + 1 < 4 else None)
        for tb in range(NBLK):
            ps = psp.tile([P, 512], F32, tag="psp")
            terms = ((xh, wvh_), (xl, wvh_), (xh, wvl_))
            n = 0
            for xt_, wt in terms:
                for dp in range(4):
                    nc.tensor.matmul(
                        ps[:, 0:256], xt_[:, 2 * dp:2 * dp + 2, tb * P:(tb + 1) * P],
                        wt[:, 2 * dp:2 * dp + 2, :],
                        start=(n == 0), stop=(n == 11), perf_mode=DR)
                    n += 1
            dst_h = vh[:, tb, vq * 256:(vq + 1) * 256]
            dst_l = vl[:, tb, vq * 256:(vq + 1) * 256]
            nc.scalar.copy(dst_h, ps[:, 0:256])
            nc.gpsimd.tensor_sub(dst_l, ps[:, 0:256], dst_h)

    # frees 3 PSUM banks for psO/psD below; psp is kept: it is reused for
    # the transposed-score tiles so the attention phase needs no PSUM
    # pool handover on the score path.
    for pool in (pst, wvpool, wqpool, wvfpool, wfpool, xpool):
        pool.release()

    # ---------------- phase 2: attention ----------------
    psO = tc.alloc_tile_pool(name="psO", bufs=2, space="PSUM")
    psD = tc.alloc_tile_pool(name="psD", bufs=2, space="PSUM")
    pth_pool = tc.alloc_tile_pool(name="pth", bufs=10)
    ptl_pool = tc.alloc_tile_pool(name="ptl", bufs=10)
    p32_pool = tc.alloc_tile_pool(name="p32", bufs=4)
    posb = tc.alloc_tile_pool(name="posb", bufs=2)
    sden = tc.alloc_tile_pool(name="sden", bufs=4)

    for jn in range(NJ):
        nmb = 4 * jn + 4          # m-blocks in this chunk
        pth, ptl = {}, {}
        # scores (transposed) + exp + P hi/lo quant, per m-block
        for mb in range(nmb):
            il = mb - 4 * jn      # >= 0 on the diagonal chunk blocks
            diag = il >= 0
            n0r = 128 * il if diag else 0
            w = 512 - n0r
            p_idx, sl = mb // 2, mb % 2
            if sl == 0:
                pth[p_idx] = pth_pool.tile([P, 2, 512], F8, tag="pth",
                                           name="pth")
                ptl[p_idx] = ptl_pool.tile([P, 2, 512], F8, tag="ptl",
                                           name="ptl")
            if n0r > 0:
                # causally-dead region of the pair tile; AV reads it as 0
                nc.gpsimd.memset(pth[p_idx][:, sl, 0:n0r], 0.0)
                nc.gpsimd.memset(ptl[p_idx][:, sl, 0:n0r], 0.0)
            ps = psp.tile([P, 512], F32, tag="psp")
            for dp in range(4):
                nc.tensor.matmul(
                    ps[:, 0:w], kT[:, 2 * dp:2 * dp + 2, mb * P:(mb + 1) * P],
                    qT[:, 2 * dp:2 * dp + 2,
                       jn * 512 + n0r:(jn + 1) * 512],
                    start=(dp == 0), stop=(dp == 3), perf_mode=DR)
            if diag:
                nc.vector.tensor_add(ps[:, 0:P], ps[:, 0:P], tri)
            p32 = p32_pool.tile([P, 512], BF16, tag="p32")
            nc.scalar.activation(p32[:, 0:w], ps[:, 0:w], EXP, scale=ESCALE)
            nc.vector.tensor_copy(pth[p_idx][:, sl, n0r:512], p32[:, 0:w])
            nc.gpsimd.tensor_sub(ptl[p_idx][:, sl, n0r:512], p32[:, 0:w],
                                 pth[p_idx][:, sl, n0r:512])

        # AV + den, per 128-query block
        for nb in range(4):
            pmax = (4 * jn + nb) // 2
            O = psO.tile([P, D], F32, tag="psO")
            dps = psD.tile([P, 4], F32, tag="psD")
            for p in range(pmax + 1):
                last = p == pmax
                terms = ((pth[p], vh), (ptl[p], vh), (pth[p], vl))
                for t, (pt, vv) in enumerate(terms):
                    for dh in range(2):
                        nc.tensor.matmul(
                            O[:, dh * 512:(dh + 1) * 512],
                            pt[:, :, nb * P:(nb + 1) * P],
                            vv[:, 2 * p:2 * p + 2, dh * 512:(dh + 1) * 512],
                            start=(p == 0 and t == 0), stop=(last and t == 2),
                            perf_mode=DR)
                nc.tensor.matmul(dps[:, 0:2], pth[p][:, :, nb * P:(nb + 1) * P],
                                 ones8, start=(p == 0), stop=False, perf_mode=DR)
                nc.tensor.matmul(dps[:, 0:2], ptl[p][:, :, nb * P:(nb + 1) * P],
                                 ones8, start=False, stop=last, perf_mode=DR)
            d2 = sden.tile([P, 4], F32, tag="d2")
            nc.vector.tensor_scalar_mul(d2[:, 1:2], dps[:, 0:1], WSCALE)
            nc.vector.reciprocal(d2[:, 2:3], d2[:, 1:2])
            Osb = posb.tile([P, D], F32, tag="Osb")
            nc.scalar.mul(Osb, O, d2[:, 2:3])
            i = 4 * jn + nb
            nc.sync.dma_start(out=out[i * P:(i + 1) * P, :], in_=Osb)

    for pool in (sden, posb, p32_pool, ptl_pool, pth_pool,
                 psD, psO, psp, res, const):
        pool.release()


_NC_CACHE = None


def _get_nc():
    global _NC_CACHE
    if _NC_CACHE is None:
        _NC_CACHE = build_program()
    return _NC_CACHE


def kernel(x, W_query, W_key, W_value):
    """Full causal attention: x [8, 2048, 1024] -> [8, 2048, 1024] (f32)."""
    nc = _get_nc()
    x = np.ascontiguousarray(np.asarray(x, dtype=np.float32))
    wq = np.ascontiguousarray(np.asarray(W_query, dtype=np.float32))
    wk = np.ascontiguousarray(np.asarray(W_key, dtype=np.float32))
    wv = np.ascontiguousarray(np.asarray(W_value, dtype=np.float32))
    n_cores = x.shape[0]
    in_maps = [
        {"x": x[b], "W_query": wq, "W_key": wk, "W_value": wv}
        for b in range(n_cores)
    ]
    res = bass_utils.run_bass_kernel_spmd(nc, in_maps, core_ids=list(range(n_cores)))
    return np.stack([res.results[b]["out"] for b in range(n_cores)])


# revision 18
# speedup vs baseline: 1.0978x; 1.0978x over previous
"""Causal attention kernel for Trainium2 (Bass/Tile), 8-core data-parallel.

Problem: x [8, 2048, 1024] f32; W_query/W_key/W_value [1024, 1024] f32.
    q = x @ Wq; k = x @ Wk; v = x @ Wv       (per batch element)
    out = softmax(causal(q k^T) / 32) @ v

Sharding: batch dim (8) across the 8 NeuronCores, one batch element per
core; each core runs the identical single-core program on its slice.

All heavy matmuls run in fp8 e4m3 with MatmulPerfMode.DoubleRow, which
contracts 256 rows per instruction at 0.5 cycles/moving-row (4x the
fp32r full rate).  fp8's ~2.6% elementwise quantization error is kept
out of the signal path with 2-term hi/lo splits (residual also in fp8):

  x  ~ x_hi + x_lo            W' = 32*W ~ W_hi + W_lo  (scaled out of
                                                        e4m3 subnormals)
  proj (3 terms):  p = x_hi@W_hi + x_lo@W_hi + x_hi@W_lo   (f32 PSUM)
  qT, kT = fp8(p)  (single; the only uncompensated quantization -
                    it only perturbs logits, ~1.1% output error)
  v ~ v_hi + v_lo
  scores computed TRANSPOSED: S^T[m,n] = kT-block (stationary) x qT
  (moving), so exp(S^T) IS P^T - no PE transposes of P needed.
  P ~ P_hi + P_lo (from bf16 exp staging)
  AV (3 terms): O = P_hi@v_hi + P_lo@v_hi + P_hi@v_lo    (f32 PSUM)
  den via tiny fp8 ones-matmuls accumulated alongside; out = O/(32 den)

Measured end-to-end (numpy bit-sim of this exact arithmetic):
rel err ~1.2e-2.  The exp scale is 1/(32*32*32): 1/sqrt(d) plus the
two W scalings.
"""

import os

import numpy as np

# Defensive: recover wedged cores at NRT/PJRT init (no-op on healthy devices).
os.environ.setdefault("NEURON_RT_RESET_CORES", "1")

import concourse.tile as tile
import concourse.mybir as mybir
from concourse import bacc, bass_utils
from concourse.masks import make_identity

F32 = mybir.dt.float32
F32R = mybir.dt.float32r
F8 = mybir.dt.float8e4
BF16 = mybir.dt.bfloat16
EXP = mybir.ActivationFunctionType.Exp
DR = mybir.MatmulPerfMode.DoubleRow
MULT = mybir.AluOpType.mult
SUB = mybir.AluOpType.subtract

NTOK = 2048      # tokens per batch element (= per core)
D = 1024         # d_in = d_out
P = 128          # partitions
DC = D // P      # 8 d-chunks
NBLK = NTOK // P     # 16 token blocks
NJ = NTOK // 512     # 4 query chunks of 512
NEG = -1.0e9
ESCALE = 1.0 / (32.0 * 32.0 * 32.0)  # exp scale: 1/sqrt(d) and two 32x W scales
WSCALE = 32.0


def build_program():
    nc = bacc.Bacc("TRN2", target_bir_lowering=False, debug=False,
                   num_devices=8)
    x = nc.dram_tensor("x", [NTOK, D], F32, kind="ExternalInput").ap()
    wq = nc.dram_tensor("W_query", [D, D], F32, kind="ExternalInput").ap()
    wk = nc.dram_tensor("W_key", [D, D], F32, kind="ExternalInput").ap()
    wv = nc.dram_tensor("W_value", [D, D], F32, kind="ExternalInput").ap()
    out = nc.dram_tensor("out", [NTOK, D], F32, kind="ExternalOutput").ap()

    with tile.TileContext(nc) as tc:
        _emit(nc, tc, x, wq, wk, wv, out)
    nc.compile()
    return nc


def _emit(nc, tc, x, wq, wk, wv, out):
    const = tc.alloc_tile_pool(name="const", bufs=1)
    res = tc.alloc_tile_pool(name="res", bufs=1)

    # constants
    id32 = const.tile([P, P], F32, tag="id32")
    make_identity(nc, id32)
    id32r = const.tile([P, P], F32R, tag="id32r")
    nc.vector.tensor_copy(id32r, id32)
    # additive causal triangle for transposed scores: T[p, f] = 0 where
    # f >= p (valid, n >= m), NEG where f < p.
    tri = const.tile([P, P], F32, tag="tri")
    nc.vector.memset(tri, 0.0)
    nc.gpsimd.affine_select(
        out=tri, in_=tri, compare_op=mybir.AluOpType.is_ge, fill=NEG,
        base=0, pattern=[[1, P]], channel_multiplier=-1)
    ones8 = const.tile([P, 2, 2], F8, tag="ones8")
    nc.vector.memset(ones8, 1.0)

    # fp8 residents
    xh = res.tile([P, DC, NTOK], F8, tag="xh")   # [d_in%128, dc, tok]
    xl = res.tile([P, DC, NTOK], F8, tag="xl")
    kT = res.tile([P, DC, NTOK], F8, tag="kT")   # [d_out%128, dc, m]
    qT = res.tile([P, DC, NTOK], F8, tag="qT")   # [d_out%128, dc, n]
    vh = res.tile([P, NBLK, D], F8, tag="vh")    # [m%128, mblock, d]
    vl = res.tile([P, NBLK, D], F8, tag="vl")
    wkh = res.tile([P, DC, D], F8, tag="wkh")    # [d_in%128, dc, d_out]
    wkl = res.tile([P, DC, D], F8, tag="wkl")

    # ---------------- phase 1: projections ----------------
    # x/out DMAs issue from SP; W DMAs from ACT so neither stream blocks
    # the other at the issue queue.
    # psS (attention scores) is allocated first so its 2 banks never
    # overlap the phase-1 PSUM pools: the first score matmuls need not
    # wait for projection-psum drains.  Pools release in stack order.
    psS = tc.alloc_tile_pool(name="psS", bufs=2, space="PSUM")
    xpool = tc.alloc_tile_pool(name="xpool", bufs=4)    # x f32 staging
    wfpool = tc.alloc_tile_pool(name="wfpool", bufs=5)  # W f32 staging (k/q)
    wvfpool = tc.alloc_tile_pool(name="wvfpool", bufs=2)  # W f32 staging (v)
    wqpool = tc.alloc_tile_pool(name="wqpool", bufs=3)  # Wq fp8 panels
    wvpool = tc.alloc_tile_pool(name="wvpool", bufs=2)  # Wv fp8 quarters
    pst = tc.alloc_tile_pool(name="pst", bufs=2, space="PSUM")
    psp = tc.alloc_tile_pool(name="psp", bufs=4, space="PSUM")

    def dma_w_panel(w_ap, pc):
        wf = wfpool.tile([P, DC, P], F32, tag="wf")
        nc.scalar.dma_start(
            out=wf,
            in_=w_ap[:, pc * P:(pc + 1) * P].rearrange("(c p) f -> p c f", p=P))
        return wf

    def quant_w(wf, hi_ap, lo_ap):
        # hi = fp8(32*W); lo = fp8(32*W - hi)
        nc.scalar.mul(hi_ap, wf, WSCALE)
        nc.vector.scalar_tensor_tensor(
            out=lo_ap, in0=wf, scalar=WSCALE, in1=hi_ap, op0=MULT, op1=SUB)

    def transpose_block(tbg):
        # x token block [128, 1024] -> xT hi/lo [:, :, tbg*128:+128]
        xt = xpool.tile([P, D], F32R, tag="xt")
        nc.sync.dma_start(out=xt, in_=x[tbg * P:(tbg + 1) * P, :].bitcast(F32R))
        for g in range(2):
            trp = pst.tile([P, 512], F32R, tag="trp")
            for b4 in range(4):
                nc.tensor.transpose(
                    trp[:, b4 * P:(b4 + 1) * P],
                    xt[:, (g * 4 + b4) * P:(g * 4 + b4 + 1) * P], id32r)
            src = trp.bitcast(F32).rearrange("p (b f) -> p b f", b=4)
            dst_h = xh[:, g * 4:(g + 1) * 4, tbg * P:(tbg + 1) * P]
            dst_l = xl[:, g * 4:(g + 1) * 4, tbg * P:(tbg + 1) * P]
            nc.vector.tensor_copy(dst_h, src)
            nc.gpsimd.tensor_sub(dst_l, src, dst_h)

    def proj_kq(dst, whi, wlo, pc, jc, pslice, requant_pool):
        # one [d_out 128, tok 512] panel-chunk of q/k projection
        ps = psp.tile([P, 512], F32, tag="psp")
        terms = ((whi, xh), (whi, xl), (wlo, xh))
        n = 0
        for wt, xt_ in terms:
            for dp in range(4):
                nc.tensor.matmul(
                    ps, wt[:, 2 * dp:2 * dp + 2, pslice],
                    xt_[:, 2 * dp:2 * dp + 2, jc * 512:(jc + 1) * 512],
                    start=(n == 0), stop=(n == 11), perf_mode=DR)
                n += 1
        if requant_pool:
            nc.gpsimd.tensor_copy(dst[:, pc, jc * 512:(jc + 1) * 512], ps)
        else:
            nc.scalar.copy(dst[:, pc, jc * 512:(jc + 1) * 512], ps)

    # k-projection: chunk-outer, with the x transposes of the NEXT chunk
    # interleaved between panels so the transpose->quant chain drains
    # while proj matmuls run.  Wk panels quantize one ahead of first use.
    def quant_wk(pc):
        quant_w(wk_f32[pc], wkh[:, :, pc * P:(pc + 1) * P],
                wkl[:, :, pc * P:(pc + 1) * P])

    wk_f32 = []
    for jc in range(NJ):
        if jc == 0:
            for tbg in range(4):
                transpose_block(tbg)
            wk_f32 = [dma_w_panel(wk, pc) for pc in range(5)]
            quant_wk(0)
        for pc in range(DC):
            if jc == 0:
                if pc + 5 < DC:
                    wk_f32.append(dma_w_panel(wk, pc + 5))
                if pc + 1 < DC:
                    quant_wk(pc + 1)
            proj_kq(kT, wkh, wkl, pc, jc,
                    slice(pc * P, (pc + 1) * P), requant_pool=True)
            if jc < NJ - 1 and pc % 2 == 0:
                transpose_block(4 * (jc + 1) + pc // 2)

    # q-proj: stream Wq panels with one-panel prefetch
    def load_wq(pc):
        wf = dma_w_panel(wq, pc)
        wqh = wqpool.tile([P, DC, P], F8, tag="wqh", name="wqh")
        wql = wqpool.tile([P, DC, P], F8, tag="wql", name="wql")
        quant_w(wf, wqh, wql)
        return wqh, wql

    nxt = load_wq(0)
    for pc in range(DC):
        cur, nxt = nxt, (load_wq(pc + 1) if pc + 1 < DC else None)
        for jc in range(NJ):
            proj_kq(qT, cur[0], cur[1], pc, jc,
                    slice(0, P), requant_pool=False)

    # v-proj: stream Wv in 256-wide d_out quarters with prefetch;
    # v needs a hi+lo split (ACT hi, Pool lo-sub)
    def load_wv(vq):
        wvf = wvfpool.tile([P, DC, 256], F32, tag="wvf")
        nc.scalar.dma_start(
            out=wvf,
            in_=wv[:, vq * 256:(vq + 1) * 256].rearrange("(c p) f -> p c f", p=P))
        wvh_ = wvpool.tile([P, DC, 256], F8, tag="wvh", name="wvh")
        wvl_ = wvpool.tile([P, DC, 256], F8, tag="wvl", name="wvl")
        quant_w(wvf, wvh_, wvl_)
        return wvh_, wvl_

    nxtv = load_wv(0)
    for vq in range(4):
        (wvh_, wvl_), nxtv = nxtv, (load_wv(vq + 1) if vq + 1 < 4 else None)
        for tb in range(NBLK):
            ps = psp.tile([P, 512], F32, tag="psp")
            terms = ((xh, wvh_), (xl, wvh_), (xh, wvl_))
            n = 0
            for xt_, wt in terms:
                for dp in range(4):
                    nc.tensor.matmul(
                        ps[:, 0:256], xt_[:, 2 * dp:2 * dp + 2, tb * P:(tb + 1) * P],
                        wt[:, 2 * dp:2 * dp + 2, :],
                        start=(n == 0), stop=(n == 11), perf_mode=DR)
                    n += 1
            dst_h = vh[:, tb, vq * 256:(vq + 1) * 256]
            dst_l = vl[:, tb, vq * 256:(vq + 1) * 256]
            nc.scalar.copy(dst_h, ps[:, 0:256])
            nc.gpsimd.tensor_sub(dst_l, ps[:, 0:256], dst_h)

    # frees 6 PSUM banks for psO/psD below; psS keeps its own 2 banks so
    # the attention score path has no PSUM handover.
    for pool in (psp, pst, wvpool, wqpool, wvfpool, wfpool, xpool):
        pool.release()

    # ---------------- phase 2: attention ----------------
    psO = tc.alloc_tile_pool(name="psO", bufs=2, space="PSUM")
    psD = tc.alloc_tile_pool(name="psD", bufs=2, space="PSUM")
    pth_pool = tc.alloc_tile_pool(name="pth", bufs=10)
    ptl_pool = tc.alloc_tile_pool(name="ptl", bufs=10)
    p32_pool = tc.alloc_tile_pool(name="p32", bufs=4)
    posb = tc.alloc_tile_pool(name="posb", bufs=2)
    sden = tc.alloc_tile_pool(name="sden", bufs=4)

    for jn in range(NJ):
        nmb = 4 * jn + 4          # m-blocks in this chunk
        pth, ptl = {}, {}
        # scores (transposed) + exp + P hi/lo quant, per m-block
        for mb in range(nmb):
            il = mb - 4 * jn      # >= 0 on the diagonal chunk blocks
            diag = il >= 0
            n0r = 128 * il if diag else 0
            w = 512 - n0r
            p_idx, sl = mb // 2, mb % 2
            if sl == 0:
                pth[p_idx] = pth_pool.tile([P, 2, 512], F8, tag="pth",
                                           name="pth")
                ptl[p_idx] = ptl_pool.tile([P, 2, 512], F8, tag="ptl",
                                           name="ptl")
            if n0r > 0:
                # causally-dead region of the pair tile; AV reads it as 0
                nc.gpsimd.memset(pth[p_idx][:, sl, 0:n0r], 0.0)
                nc.gpsimd.memset(ptl[p_idx][:, sl, 0:n0r], 0.0)
            ps = psS.tile([P, 512], F32, tag="psS")
            for dp in range(4):
                nc.tensor.matmul(
                    ps[:, 0:w], kT[:, 2 * dp:2 * dp + 2, mb * P:(mb + 1) * P],
                    qT[:, 2 * dp:2 * dp + 2,
                       jn * 512 + n0r:(jn + 1) * 512],
                    start=(dp == 0), stop=(dp == 3), perf_mode=DR)
            if diag:
                nc.vector.tensor_add(ps[:, 0:P], ps[:, 0:P], tri)
            p32 = p32_pool.tile([P, 512], BF16, tag="p32")
            nc.scalar.activation(p32[:, 0:w], ps[:, 0:w], EXP, scale=ESCALE)
            nc.vector.tensor_copy(pth[p_idx][:, sl, n0r:512], p32[:, 0:w])
            nc.gpsimd.tensor_sub(ptl[p_idx][:, sl, n0r:512], p32[:, 0:w],
                                 pth[p_idx][:, sl, n0r:512])

        # AV + den, per 128-query block
        for nb in range(4):
            pmax = (4 * jn + nb) // 2
            O = psO.tile([P, D], F32, tag="psO")
            dps = psD.tile([P, 4], F32, tag="psD")
            for p in range(pmax + 1):
                last = p == pmax
                terms = ((pth[p], vh), (ptl[p], vh), (pth[p], vl))
                for t, (pt, vv) in enumerate(terms):
                    for dh in range(2):
                        nc.tensor.matmul(
                            O[:, dh * 512:(dh + 1) * 512],
                            pt[:, :, nb * P:(nb + 1) * P],
                            vv[:, 2 * p:2 * p + 2, dh * 512:(dh + 1) * 512],
                            start=(p == 0 and t == 0), stop=(last and t == 2),
                            perf_mode=DR)
                nc.tensor.matmul(dps[:, 0:2], pth[p][:, :, nb * P:(nb + 1) * P],
                                 ones8, start=(p == 0), stop=False, perf_mode=DR)
                nc.tensor.matmul(dps[:, 0:2], ptl[p][:, :, nb * P:(nb + 1) * P],
                                 ones8, start=False, stop=last, perf_mode=DR)
            d2 = sden.tile([P, 4], F32, tag="d2")
            nc.vector.tensor_scalar_mul(d2[:, 1:2], dps[:, 0:1], WSCALE)
            nc.vector.reciprocal(d2[:, 2:3], d2[:, 1:2])
            Osb = posb.tile([P, D], F32, tag="Osb")
            nc.scalar.mul(Osb, O, d2[:, 2:3])
            i = 4 * jn + nb
            nc.sync.dma_start(out=out[i * P:(i + 1) * P, :], in_=Osb)

    for pool in (sden, posb, p32_pool, ptl_pool, pth_pool,
                 psD, psO, psS, res, const):
        pool.release()


_NC_CACHE = None


def _get_nc():
    global _NC_CACHE
    if _NC_CACHE is None:
        _NC_CACHE = build_program()
    return _NC_CACHE


def kernel(x, W_query, W_key, W_value):
    """Full causal attention: x [8, 2048, 1024] -> [8, 2048, 1024] (f32)."""
    nc = _get_nc()
    x = np.ascontiguousarray(np.asarray(x, dtype=np.float32))
    wq = np.ascontiguousarray(np.asarray(W_query, dtype=np.float32))
    wk = np.ascontiguousarray(np.asarray(W_key, dtype=np.float32))
    wv = np.ascontiguousarray(np.asarray(W_value, dtype=np.float32))
    n_cores = x.shape[0]
    in_maps = [
        {"x": x[b], "W_query": wq, "W_key": wk, "W_value": wv}
        for b in range(n_cores)
    ]
    res = bass_utils.run_bass_kernel_spmd(nc, in_maps, core_ids=list(range(n_cores)))
    return np.stack([res.results[b]["out"] for b in range(n_cores)])
